# revision 1
# baseline (speedup 1.0000x reference)
"""Trainium2 Bass kernel for batched multi-head attention that returns
(out, weights) like the reference nn.Module.

Problem: B=2, H=16, S=4096, D=64, causal mask, Temp=1.0, scale 1/sqrt(64).
reference returns (out [B,H,S,D], weights [B,H,S,S]) in float32.

Strategy (8 NeuronCores, head-parallel):
  - B*H = 32 (batch, head) pairs -> 4 per core. No collectives needed.
  - Per head, two passes sharing Q^T / K^T (bf16, built once via PE
    transposes):
      Phase 1 (K-major): S^T tiles = K^T.T @ Q^T -> exp -> PV matmul with a
        ones column appended to V gives both U^T = V^T @ expS^T and the
        softmax row sums. Transpose U^T back (tiny), reciprocal, normalize,
        write `out`.
      Phase 2 (Q-major): recompute score tiles Q^T.T @ K^T (bit-identical
        contraction), exp -> bf16, multiply by the phase-1 reciprocal
        (per-partition scalar), write `weights` rows.
  - Causal structure: strictly-upper-triangular blocks are never computed or
    written; the runtime zero-fills output buffers (donated zero buffers in
    the PJRT path), so those weights are exactly 0 like the reference.
  - Diagonal 128x128 blocks get a -8e9 additive mask const before the exp
    (exp scale is 0.125 = 1/sqrt(64), so -8e9*0.125 = -1e9, underflows to 0).

Host side: inspects Mask. Strict-upper-triangular -> causal kernel;
all-zeros -> non-causal kernel; anything else -> numpy fallback (slow but
correct).
"""

import numpy as np
from contextlib import ExitStack

import concourse.bass as bass
import concourse.bacc as bacc
import concourse.mybir as mybir
import concourse.tile as tile
from concourse.bass_utils import run_bass_kernel_spmd

F32 = mybir.dt.float32
BF16 = mybir.dt.bfloat16
Exp = mybir.ActivationFunctionType.Exp

B, H, S, D = 2, 16, 4096, 64
P = 128              # partition tile (q/k tile size)
CHUNK = 1024         # q columns per PSUM score tile (2 banks)
N_CORES = 8
HPC = (B * H) // N_CORES  # heads per core = 4
MASK_VAL = -8.0e9    # becomes -1e9 after the 0.125 exp scale
SCALE = 0.125        # 1/sqrt(64), Temp=1.0


def build_attention(causal: bool, s: int = S, hpc: int = HPC):
    """Build the per-core Bass program. Each core processes `hpc` heads of
    shape [s, D] with full K/V (no cross-core traffic)."""
    nt = s // P          # k/q tiles per head
    nch = s // CHUNK     # chunks per head
    tpch = CHUNK // P    # q-tiles per chunk (8)

    nc = bacc.Bacc("TRN2", target_bir_lowering=False, debug=False)
    q_in = nc.dram_tensor("query", [hpc, s, D], F32, kind="ExternalInput")
    k_in = nc.dram_tensor("key", [hpc, s, D], F32, kind="ExternalInput")
    v_in = nc.dram_tensor("value", [hpc, s, D], F32, kind="ExternalInput")
    w_out = nc.dram_tensor("weights", [hpc, s, s], F32, kind="ExternalOutput")
    o_out = nc.dram_tensor("out", [hpc, s, D], F32, kind="ExternalOutput")

    with tile.TileContext(nc) as tc, ExitStack() as ctx:
        consts = ctx.enter_context(tc.tile_pool(name="consts", bufs=1))
        ld = ctx.enter_context(tc.tile_pool(name="ld", bufs=3))
        qtp = ctx.enter_context(tc.tile_pool(name="qtp", bufs=2))
        ktp = ctx.enter_context(tc.tile_pool(name="ktp", bufs=2))
        vbp = ctx.enter_context(tc.tile_pool(name="vbp", bufs=2))
        expp = ctx.enter_context(tc.tile_pool(name="expp", bufs=3))
        uop = ctx.enter_context(tc.tile_pool(name="uop", bufs=2))
        uotp = ctx.enter_context(tc.tile_pool(name="uotp", bufs=2))
        recp = ctx.enter_context(tc.tile_pool(name="recp", bufs=2))
        otsp = ctx.enter_context(tc.tile_pool(name="otsp", bufs=2))
        wbp = ctx.enter_context(tc.tile_pool(name="wbp", bufs=3))
        wfp = ctx.enter_context(tc.tile_pool(name="wfp", bufs=4))
        psS = ctx.enter_context(tc.tile_pool(name="psS", bufs=2, space="PSUM"))
        psO = ctx.enter_context(tc.tile_pool(name="psO", bufs=1, space="PSUM"))
        psSm = ctx.enter_context(tc.tile_pool(name="psSm", bufs=2, space="PSUM"))

        ident = consts.tile([P, P], F32)
        nc.gpsimd.memset(ident[:], 0.0)
        nc.gpsimd.affine_select(
            out=ident[:], in_=ident[:], compare_op=mybir.AluOpType.not_equal,
            fill=1.0, base=0, pattern=[[-1, P]], channel_multiplier=1,
        )
        if causal:
            # K-major diag mask: tile is [k partition, q free]; mask where q<k.
            cm_km = consts.tile([P, P], F32)
            nc.gpsimd.memset(cm_km[:], 0.0)
            nc.gpsimd.affine_select(
                out=cm_km[:], in_=cm_km[:], compare_op=mybir.AluOpType.is_ge,
                fill=MASK_VAL, base=0, pattern=[[1, P]], channel_multiplier=-1,
            )
            # Q-major diag mask: tile is [q partition, k free]; mask where k>q.
            cm_qm = consts.tile([P, P], F32)
            nc.gpsimd.memset(cm_qm[:], 0.0)
            nc.gpsimd.affine_select(
                out=cm_qm[:], in_=cm_qm[:], compare_op=mybir.AluOpType.is_ge,
                fill=MASK_VAL, base=0, pattern=[[-1, P]], channel_multiplier=1,
            )

        for h in range(hpc):
            # ---- prep: load Q/K/V, build Q^T, K^T (bf16), V|1 (bf16) ----
            qf = ld.tile([P, nt * D], F32, tag="ld")
            nc.sync.dma_start(
                out=qf[:].rearrange("p (t d) -> p t d", d=D),
                in_=q_in[h].rearrange("(t p) d -> p t d", p=P))
            kf = ld.tile([P, nt * D], F32, tag="ld")
            nc.sync.dma_start(
                out=kf[:].rearrange("p (t d) -> p t d", d=D),
                in_=k_in[h].rearrange("(t p) d -> p t d", p=P))
            vf = ld.tile([P, nt * D], F32, tag="ld")
            nc.sync.dma_start(
                out=vf[:].rearrange("p (t d) -> p t d", d=D),
                in_=v_in[h].rearrange("(t p) d -> p t d", p=P))

            qt = qtp.tile([D, s], BF16)
            kt = ktp.tile([D, s], BF16)
            for t in range(nt):
                pq = psSm.tile([D, P], F32, tag="psSm")
                nc.tensor.transpose(pq[:], qf[:, t * D:(t + 1) * D], ident[:])
                nc.vector.tensor_copy(qt[0:D, t * P:(t + 1) * P], pq[:])
                pk = psSm.tile([D, P], F32, tag="psSm")
                nc.tensor.transpose(pk[:], kf[:, t * D:(t + 1) * D], ident[:])
                nc.vector.tensor_copy(kt[0:D, t * P:(t + 1) * P], pk[:])

            vb = vbp.tile([P, nt * (D + 1)], BF16)
            nc.vector.memset(vb[:], 1.0)
            for t in range(nt):
                nc.vector.tensor_copy(
                    vb[:, t * (D + 1):t * (D + 1) + D],
                    vf[:, t * D:(t + 1) * D])

            rec = recp.tile([P, nt], F32)

            for c in range(nch):
                # ================= phase 1 (K-major) for q-chunk c =========
                jmax = (tpch * c + tpch - 1) if causal else (nt - 1)
                po = psO.tile([D + 1, CHUNK], F32)
                for j in range(jmax + 1):
                    w_off = max(0, (j - tpch * c)) * P if causal else 0
                    pss = psS.tile([P, CHUNK], F32, tag="psS")
                    for s0 in range(0, CHUNK, 512):
                        a, b = max(w_off, s0), s0 + 512
                        if a >= b:
                            continue
                        nc.tensor.matmul(
                            pss[:, a:b], kt[0:D, j * P:(j + 1) * P],
                            qt[0:D, c * CHUNK + a:c * CHUNK + b],
                            start=True, stop=True)
                    if causal and j >= tpch * c:
                        nc.vector.tensor_add(
                            pss[:, w_off:w_off + P], pss[:, w_off:w_off + P],
                            cm_km[:])
                    eT = expp.tile([P, CHUNK], BF16, tag="expp")
                    nc.scalar.activation(eT[:, w_off:], pss[:, w_off:], Exp,
                                         scale=SCALE)
                    for s0 in range(0, CHUNK, 512):
                        a, b = max(w_off, s0), s0 + 512
                        if a >= b:
                            continue
                        nc.tensor.matmul(
                            po[:, a:b], vb[:, j * (D + 1):(j + 1) * (D + 1)],
                            eT[:, a:b], start=(j == 0), stop=(j == jmax),
                            skip_group_check=True)
                # U^T [65, CHUNK] -> per q-tile: transpose, recip, write out
                uo = uop.tile([D + 1, CHUNK], F32)
                nc.vector.tensor_copy(uo[:], po[:])
                for t in range(tpch):
                    i = tpch * c + t
                    pot = psSm.tile([P, D + 1], F32, tag="psSm")
                    nc.tensor.transpose(
                        pot[:], uo[0:D + 1, t * P:(t + 1) * P],
                        ident[0:D + 1, 0:D + 1])
                    ut = uotp.tile([P, D + 1], F32)
                    nc.vector.tensor_copy(ut[:], pot[:])
                    nc.vector.reciprocal(rec[:, i:i + 1], ut[:, D:D + 1])
                    ots = otsp.tile([P, D], F32)
                    nc.vector.tensor_scalar_mul(ots[:], ut[:, 0:D],
                                                rec[:, i:i + 1])
                    nc.sync.dma_start(out=o_out[h, i * P:(i + 1) * P, :],
                                      in_=ots[:])
                # ================= phase 2 (Q-major) for q-tiles of chunk c =
                for t in range(tpch):
                    i = tpch * c + t
                    w = (i + 1) * P if causal else s
                    for c2 in range((w + CHUNK - 1) // CHUNK):
                        base = c2 * CHUNK
                        wc = min(CHUNK, w - base)
                        ps2 = psS.tile([P, CHUNK], F32, tag="psS")
                        for s0 in range(0, wc, 512):
                            b = min(s0 + 512, wc)
                            nc.tensor.matmul(
                                ps2[:, s0:b], qt[0:D, i * P:(i + 1) * P],
                                kt[0:D, base + s0:base + b],
                                start=True, stop=True)
                        if causal and base + wc == w:
                            nc.vector.tensor_add(
                                ps2[:, wc - P:wc], ps2[:, wc - P:wc], cm_qm[:])
                        wb = wbp.tile([P, CHUNK], BF16, tag="wbp")
                        nc.scalar.activation(wb[:, 0:wc], ps2[:, 0:wc], Exp,
                                             scale=SCALE)
                        wf = wfp.tile([P, CHUNK], F32, tag="wfp")
                        nc.vector.tensor_scalar_mul(wf[:, 0:wc], wb[:, 0:wc],
                                                    rec[:, i:i + 1])
                        nc.sync.dma_start(
                            out=w_out[h, i * P:(i + 1) * P, base:base + wc],
                            in_=wf[:, 0:wc])
    nc.compile()
    return nc


_NC_CACHE = {}


def _get_nc(causal: bool):
    if causal not in _NC_CACHE:
        _NC_CACHE[causal] = build_attention(causal)
    return _NC_CACHE[causal]


def _host_fallback(query, key, value, Mask):
    """Numpy reference path for arbitrary masks (slow, correct)."""
    q = query.reshape(B * H, S, D)
    k = key.reshape(B * H, S, D)
    v = value.reshape(B * H, S, D)
    out = np.empty((B * H, S, D), np.float32)
    wts = np.empty((B * H, S, S), np.float32)
    m = (-1e9 * Mask).astype(np.float32)
    for i in range(B * H):
        sc = (q[i] @ k[i].T) * np.float32(SCALE) + m
        sc -= sc.max(axis=-1, keepdims=True)
        np.exp(sc, out=sc)
        sc /= sc.sum(axis=-1, keepdims=True)
        wts[i] = sc
        out[i] = sc @ v[i]
    return (out.reshape(B, H, S, D), wts.reshape(B, H, S, S))


def _classify_mask(Mask):
    m = np.asarray(Mask)
    if m.shape != (S, S):
        return "other"
    if not m.any():
        return "zeros"
    # strict upper triangular ones?
    expect = np.triu(np.ones((S, S), dtype=m.dtype), k=1)
    if np.array_equal(m, expect):
        return "causal"
    return "other"


def kernel(query, key, value, Mask):
    query = np.ascontiguousarray(np.asarray(query, dtype=np.float32))
    key = np.ascontiguousarray(np.asarray(key, dtype=np.float32))
    value = np.ascontiguousarray(np.asarray(value, dtype=np.float32))
    Mask = np.asarray(Mask, dtype=np.float32)

    kind = _classify_mask(Mask)
    if kind == "other":
        return _host_fallback(query, key, value, Mask)
    causal = kind == "causal"

    nc = _get_nc(causal)
    q = query.reshape(B * H, S, D)
    k = key.reshape(B * H, S, D)
    v = value.reshape(B * H, S, D)
    in_maps = [
        {"query": q[c * HPC:(c + 1) * HPC],
         "key": k[c * HPC:(c + 1) * HPC],
         "value": v[c * HPC:(c + 1) * HPC]}
        for c in range(N_CORES)
    ]
    res = run_bass_kernel_spmd(nc, in_maps, core_ids=list(range(N_CORES)))

    wts = np.empty((B * H, S, S), np.float32)
    out = np.empty((B * H, S, D), np.float32)
    for c in range(N_CORES):
        wts[c * HPC:(c + 1) * HPC] = res.results[c]["weights"]
        out[c * HPC:(c + 1) * HPC] = res.results[c]["out"]
    return (out.reshape(B, H, S, D), wts.reshape(B, H, S, S))


# revision 5
# speedup vs baseline: 330.9548x; 330.9548x over previous
"""Trainium2 Bass kernel for batched multi-head attention that returns
(out, weights) like the reference nn.Module.

Problem: B=2, H=16, S=4096, D=64, causal mask, Temp=1.0, scale 1/sqrt(64).
reference returns (out [B,H,S,D], weights [B,H,S,S]) in float32.

Strategy (8 NeuronCores, head-parallel):
  - B*H = 32 (batch, head) pairs -> 4 per core. No collectives needed.
  - Per head, two passes sharing Q^T / K^T (bf16, built once via PE
    transposes):
      Phase 1 (K-major): S^T tiles = K^T.T @ Q^T -> exp -> PV matmul with a
        ones column appended to V gives both U^T = V^T @ expS^T and the
        softmax row sums. Transpose U^T back (tiny), reciprocal, normalize,
        write `out`.
      Phase 2 (Q-major): recompute score tiles Q^T.T @ K^T (bit-identical
        contraction), exp -> bf16, multiply by the phase-1 reciprocal
        (per-partition scalar), write `weights` rows.
  - Causal structure: strictly-upper-triangular blocks are never computed or
    written; the runtime zero-fills output buffers (donated zero buffers in
    the PJRT path), so those weights are exactly 0 like the reference.
  - Diagonal 128x128 blocks get a -8e9 additive mask const before the exp
    (exp scale is 0.125 = 1/sqrt(64), so -8e9*0.125 = -1e9, underflows to 0).

Host side: inspects Mask. Strict-upper-triangular -> causal kernel;
all-zeros -> non-causal kernel; anything else -> numpy fallback (slow but
correct).
"""

import numpy as np
from contextlib import ExitStack

import concourse.bass as bass
import concourse.bacc as bacc
import concourse.mybir as mybir
import concourse.tile as tile
from concourse.bass_utils import run_bass_kernel_spmd

F32 = mybir.dt.float32
BF16 = mybir.dt.bfloat16
Exp = mybir.ActivationFunctionType.Exp

B, H, S, D = 2, 16, 4096, 64
P = 128              # partition tile (q/k tile size)
CHUNK = 1024         # q columns per PSUM score tile (2 banks)
N_CORES = 8
HPC = (B * H) // N_CORES  # heads per core = 4
MASK_VAL = -8.0e9    # becomes -1e9 after the 0.125 exp scale
SCALE = 0.125        # 1/sqrt(64), Temp=1.0


def build_attention(causal: bool, s: int = S, hpc: int = HPC,
                    bench_reps: int = 0):
    """Build the per-core Bass program. Each core processes `hpc` heads of
    shape [s, D] with full K/V (no cross-core traffic).

    bench_reps > 0 builds a benchmark variant: the big outputs become
    internal DRAM scratch (nothing large crosses PJRT) and the whole body
    repeats bench_reps times inside a hardware For_i loop."""
    nt = s // P          # k/q tiles per head
    nch = s // CHUNK     # chunks per head
    tpch = CHUNK // P    # q-tiles per chunk (8)
    bench = bench_reps > 0

    nc = bacc.Bacc("TRN2", target_bir_lowering=False, debug=False)
    q_in = nc.dram_tensor("query", [hpc, s, D], F32, kind="ExternalInput")
    k_in = nc.dram_tensor("key", [hpc, s, D], F32, kind="ExternalInput")
    v_in = nc.dram_tensor("value", [hpc, s, D], F32, kind="ExternalInput")
    if bench:
        w_out = nc.dram_tensor("w_scratch", [hpc, s, s], F32)
        o_out = nc.dram_tensor("o_scratch", [hpc, s, D], F32)
        done = nc.dram_tensor("done", [P, 4], F32, kind="ExternalOutput")
    else:
        w_out = nc.dram_tensor("weights", [hpc, s, s], F32, kind="ExternalOutput")
        o_out = nc.dram_tensor("out", [hpc, s, D], F32, kind="ExternalOutput")

    with tile.TileContext(nc) as tc, ExitStack() as ctx:
        consts = ctx.enter_context(tc.tile_pool(name="consts", bufs=1))
        ld = ctx.enter_context(tc.tile_pool(name="ld", bufs=3))
        qtp = ctx.enter_context(tc.tile_pool(name="qtp", bufs=2))
        ktp = ctx.enter_context(tc.tile_pool(name="ktp", bufs=2))
        vbp = ctx.enter_context(tc.tile_pool(name="vbp", bufs=2))
        expp = ctx.enter_context(tc.tile_pool(name="expp", bufs=3))
        uop = ctx.enter_context(tc.tile_pool(name="uop", bufs=2))
        uotp = ctx.enter_context(tc.tile_pool(name="uotp", bufs=2))
        recp = ctx.enter_context(tc.tile_pool(name="recp", bufs=2))
        otsp = ctx.enter_context(tc.tile_pool(name="otsp", bufs=2))
        wbp = ctx.enter_context(tc.tile_pool(name="wbp", bufs=3))
        wfp = ctx.enter_context(tc.tile_pool(name="wfp", bufs=4))
        psS = ctx.enter_context(tc.tile_pool(name="psS", bufs=2, space="PSUM"))
        psO = ctx.enter_context(tc.tile_pool(name="psO", bufs=1, space="PSUM"))
        psSm = ctx.enter_context(tc.tile_pool(name="psSm", bufs=2, space="PSUM"))

        ident = consts.tile([P, P], F32)
        nc.gpsimd.memset(ident[:], 0.0)
        nc.gpsimd.affine_select(
            out=ident[:], in_=ident[:], compare_op=mybir.AluOpType.not_equal,
            fill=1.0, base=0, pattern=[[-1, P]], channel_multiplier=1,
        )
        if causal:
            # K-major diag mask: tile is [k partition, q free]; mask where q<k.
            cm_km = consts.tile([P, P], F32)
            nc.gpsimd.memset(cm_km[:], 0.0)
            nc.gpsimd.affine_select(
                out=cm_km[:], in_=cm_km[:], compare_op=mybir.AluOpType.is_ge,
                fill=MASK_VAL, base=0, pattern=[[1, P]], channel_multiplier=-1,
            )
            # Q-major diag mask: tile is [q partition, k free]; mask where k>q.
            cm_qm = consts.tile([P, P], F32)
            nc.gpsimd.memset(cm_qm[:], 0.0)
            nc.gpsimd.affine_select(
                out=cm_qm[:], in_=cm_qm[:], compare_op=mybir.AluOpType.is_ge,
                fill=MASK_VAL, base=0, pattern=[[-1, P]], channel_multiplier=1,
            )

        def emit_head(h):
            # ---- prep: load Q/K/V, build Q^T, K^T (bf16), V|1 (bf16) ----
            qf = ld.tile([P, nt * D], F32, tag="ld")
            nc.sync.dma_start(
                out=qf[:].rearrange("p (t d) -> p t d", d=D),
                in_=q_in[h].rearrange("(t p) d -> p t d", p=P))
            kf = ld.tile([P, nt * D], F32, tag="ld")
            nc.sync.dma_start(
                out=kf[:].rearrange("p (t d) -> p t d", d=D),
                in_=k_in[h].rearrange("(t p) d -> p t d", p=P))
            vf = ld.tile([P, nt * D], F32, tag="ld")
            nc.sync.dma_start(
                out=vf[:].rearrange("p (t d) -> p t d", d=D),
                in_=v_in[h].rearrange("(t p) d -> p t d", p=P))

            qt = qtp.tile([D, s], BF16)
            kt = ktp.tile([D, s], BF16)
            for t in range(nt):
                pq = psSm.tile([D, P], F32, tag="psSm")
                nc.tensor.transpose(pq[:], qf[:, t * D:(t + 1) * D], ident[:])
                nc.vector.tensor_copy(qt[0:D, t * P:(t + 1) * P], pq[:])
                pk = psSm.tile([D, P], F32, tag="psSm")
                nc.tensor.transpose(pk[:], kf[:, t * D:(t + 1) * D], ident[:])
                nc.vector.tensor_copy(kt[0:D, t * P:(t + 1) * P], pk[:])

            vb = vbp.tile([P, nt * (D + 1)], BF16)
            nc.vector.memset(vb[:], 1.0)
            for t in range(nt):
                nc.vector.tensor_copy(
                    vb[:, t * (D + 1):t * (D + 1) + D],
                    vf[:, t * D:(t + 1) * D])

            rec = recp.tile([P, nt], F32)

            for c in range(nch):
                # ================= phase 1 (K-major) for q-chunk c =========
                jmax = (tpch * c + tpch - 1) if causal else (nt - 1)
                po = psO.tile([D + 1, CHUNK], F32)
                for j in range(jmax + 1):
                    w_off = max(0, (j - tpch * c)) * P if causal else 0
                    pss = psS.tile([P, CHUNK], F32, tag="psS")
                    for s0 in range(0, CHUNK, 512):
                        a, b = max(w_off, s0), s0 + 512
                        if a >= b:
                            continue
                        nc.tensor.matmul(
                            pss[:, a:b], kt[0:D, j * P:(j + 1) * P],
                            qt[0:D, c * CHUNK + a:c * CHUNK + b],
                            start=True, stop=True)
                    if causal and j >= tpch * c:
                        nc.vector.tensor_add(
                            pss[:, w_off:w_off + P], pss[:, w_off:w_off + P],
                            cm_km[:])
                    eT = expp.tile([P, CHUNK], BF16, tag="expp")
                    nc.scalar.activation(eT[:, w_off:], pss[:, w_off:], Exp,
                                         scale=SCALE)
                    for s0 in range(0, CHUNK, 512):
                        a, b = max(w_off, s0), s0 + 512
                        if a >= b:
                            continue
                        nc.tensor.matmul(
                            po[:, a:b], vb[:, j * (D + 1):(j + 1) * (D + 1)],
                            eT[:, a:b], start=(j == 0), stop=(j == jmax),
                            skip_group_check=True)
                # U^T [65, CHUNK] -> per q-tile: transpose, recip, write out
                uo = uop.tile([D + 1, CHUNK], F32)
                nc.vector.tensor_copy(uo[:], po[:])
                for t in range(tpch):
                    i = tpch * c + t
                    pot = psSm.tile([P, D + 1], F32, tag="psSm")
                    nc.tensor.transpose(
                        pot[:], uo[0:D + 1, t * P:(t + 1) * P],
                        ident[0:D + 1, 0:D + 1])
                    ut = uotp.tile([P, D + 1], F32)
                    nc.vector.tensor_copy(ut[:], pot[:])
                    nc.vector.reciprocal(rec[:, i:i + 1], ut[:, D:D + 1])
                    ots = otsp.tile([P, D], F32)
                    nc.vector.tensor_scalar_mul(ots[:], ut[:, 0:D],
                                                rec[:, i:i + 1])
                    nc.sync.dma_start(out=o_out[h, i * P:(i + 1) * P, :],
                                      in_=ots[:])
                # ================= phase 2 (Q-major) for q-tiles of chunk c =
                for t in range(tpch):
                    i = tpch * c + t
                    w = (i + 1) * P if causal else s
                    for c2 in range((w + CHUNK - 1) // CHUNK):
                        base = c2 * CHUNK
                        wc = min(CHUNK, w - base)
                        ps2 = psS.tile([P, CHUNK], F32, tag="psS")
                        for s0 in range(0, wc, 512):
                            b = min(s0 + 512, wc)
                            nc.tensor.matmul(
                                ps2[:, s0:b], qt[0:D, i * P:(i + 1) * P],
                                kt[0:D, base + s0:base + b],
                                start=True, stop=True)
                        if causal and base + wc == w:
                            nc.vector.tensor_add(
                                ps2[:, wc - P:wc], ps2[:, wc - P:wc], cm_qm[:])
                        wb = wbp.tile([P, CHUNK], BF16, tag="wbp")
                        nc.scalar.activation(wb[:, 0:wc], ps2[:, 0:wc], Exp,
                                             scale=SCALE)
                        wf = wfp.tile([P, CHUNK], F32, tag="wfp")
                        nc.vector.tensor_scalar_mul(wf[:, 0:wc], wb[:, 0:wc],
                                                    rec[:, i:i + 1])
                        nc.sync.dma_start(
                            out=w_out[h, i * P:(i + 1) * P, base:base + wc],
                            in_=wf[:, 0:wc])

        if bench:
            with tc.For_i(0, bench_reps, 1):
                for h in range(hpc):
                    emit_head(h)
            dn = consts.tile([P, 4], F32)
            nc.vector.memset(dn[:], 1.0)
            nc.sync.dma_start(out=done[:], in_=dn[:])
        else:
            for h in range(hpc):
                emit_head(h)
    nc.compile()
    return nc


_NC_CACHE = {}


def _get_nc(causal: bool):
    if causal not in _NC_CACHE:
        _NC_CACHE[causal] = build_attention(causal)
    return _NC_CACHE[causal]


def _host_fallback(query, key, value, Mask):
    """Numpy reference path for arbitrary masks (slow, correct)."""
    q = query.reshape(B * H, S, D)
    k = key.reshape(B * H, S, D)
    v = value.reshape(B * H, S, D)
    out = np.empty((B * H, S, D), np.float32)
    wts = np.empty((B * H, S, S), np.float32)
    m = (-1e9 * Mask).astype(np.float32)
    for i in range(B * H):
        sc = (q[i] @ k[i].T) * np.float32(SCALE) + m
        sc -= sc.max(axis=-1, keepdims=True)
        np.exp(sc, out=sc)
        sc /= sc.sum(axis=-1, keepdims=True)
        wts[i] = sc
        out[i] = sc @ v[i]
    return (out.reshape(B, H, S, D), wts.reshape(B, H, S, S))


def _classify_mask(Mask):
    m = np.asarray(Mask)
    if m.shape != (S, S):
        return "other"
    if not m.any():
        return "zeros"
    # strict upper triangular ones?
    expect = np.triu(np.ones((S, S), dtype=m.dtype), k=1)
    if np.array_equal(m, expect):
        return "causal"
    return "other"


def kernel(query, key, value, Mask):
    query = np.ascontiguousarray(np.asarray(query, dtype=np.float32))
    key = np.ascontiguousarray(np.asarray(key, dtype=np.float32))
    value = np.ascontiguousarray(np.asarray(value, dtype=np.float32))
    Mask = np.asarray(Mask, dtype=np.float32)

    kind = _classify_mask(Mask)
    if kind == "other":
        return _host_fallback(query, key, value, Mask)
    causal = kind == "causal"

    nc = _get_nc(causal)
    q = query.reshape(B * H, S, D)
    k = key.reshape(B * H, S, D)
    v = value.reshape(B * H, S, D)
    in_maps = [
        {"query": q[c * HPC:(c + 1) * HPC],
         "key": k[c * HPC:(c + 1) * HPC],
         "value": v[c * HPC:(c + 1) * HPC]}
        for c in range(N_CORES)
    ]
    res = run_bass_kernel_spmd(nc, in_maps, core_ids=list(range(N_CORES)))

    wts = np.empty((B * H, S, S), np.float32)
    out = np.empty((B * H, S, D), np.float32)
    for c in range(N_CORES):
        wts[c * HPC:(c + 1) * HPC] = res.results[c]["weights"]
        out[c * HPC:(c + 1) * HPC] = res.results[c]["out"]
    return (out.reshape(B, H, S, D), wts.reshape(B, H, S, S))
